# revision 33
# baseline (speedup 1.0000x reference)
"""Trainium2 Bass kernel for nn_Encoder_55490977464569 (binary-tree GRU encoder).

Strategy (v7)
-------------
Data-parallel over batch: B=16 -> 2 batch columns per NeuronCore, zero
collectives. Each core runs its whole tree (32767 nodes) leaves->root with all
hidden states resident in SBUF (bf16); only `targets` is streamed in.

Layout: feature-major [128 features (partitions), node*batch columns], each
level's nodes stored in BIT-REVERSED in-level order: the children of parent
tile [t0, t0+T) are child[:, t0:t0+T] and child[:, R+t0:R+t0+T], and the
parent's h writes back contiguously.

The ScalarE ACTIVATE stream is the bottleneck (every PSUM column needs one
ACT = drain+nonlinearity at 1 col/cycle @1.2GHz + ~180cy/instr overhead), so
v7 (a) minimizes ACT instructions with wide merged ACTs, (b) keeps the ACT
stream gapless via 1-deep software pipelining (emit pair i+1's front before
pair i's back) with a dependency-aware skew flush, and (c) keeps the PE HAM
clock warm (2.4GHz) with a prologue warm-up burst + keep-alive dummy matmuls,
since cold matmuls (1.2GHz) were lengthening the late-tree critical chains.

  - leaves: tanh converted to sigmoid (n = 2*sig(2pre)-1, weights doubled on
    host, fixup = one 4x-mode tensor_scalar) -> even units do ONE [2048]
    sigmoid on the 4-bank psr tile [zA|zB|nA|nB]; odd units use psZ+psN.
    Leaf units interleave 2:1 with level-13 pair fronts.
  - interior pair (2 tiles x T=512 parents): psr [rlA|rrA|rlB|rrB] -> ONE
    sigmoid [2048]; psZ [zA|zB] + psN [nA|nB] -> sigmoid + tanh [1024 each].
    W_hz/w_iz negated on host so zz = sigmoid(+pre) at scale=1.
  - tail (levels 8..0): z-gate shares the r sigmoid (3-region [rl|rr|zz]
    ACT over psr banks 0/1/2), cs fused into the previous unit's DVE stream,
    w_out in bf16 so the head matmul reads h0 directly.
"""

import sys

if "/opt/trn_rl_repo" not in sys.path:
    sys.path.insert(0, "/opt/trn_rl_repo")
if "/opt/trn_rl_repo/concourse" not in sys.path:
    sys.path.insert(0, "/opt/trn_rl_repo/concourse")

import numpy as np
import ml_dtypes

from concourse import bass, mybir, tile, bacc
from concourse import bass_utils

BF16NP = ml_dtypes.bfloat16
F32 = mybir.dt.float32
BF16 = mybir.dt.bfloat16

N_CORES = 8
DEPTH = 15
HID = 128
IN_DIM = 32
OUT_DIM = 64
BATCH = 16
B_LOCAL = BATCH // N_CORES

T_TILE = 512
SMALL_MAX_LVL = 7

ADD = mybir.AluOpType.add
SUB = mybir.AluOpType.subtract
MULT = mybir.AluOpType.mult
TANH = mybir.ActivationFunctionType.Tanh
SIGM = mybir.ActivationFunctionType.Sigmoid


def _R(l):
    return 2**l * B_LOCAL


def _bitrev(n_bits):
    n = 1 << n_bits
    p = np.zeros(n, dtype=np.int64)
    for i in range(n):
        r = 0
        x = i
        for _ in range(n_bits):
            r = (r << 1) | (x & 1)
            x >>= 1
        p[i] = r
    return p


def build_program(with_mask=False, with_bias=False):
    nc = bacc.Bacc("TRN2", target_bir_lowering=False, debug=False,
                   num_devices=1)
    leaf = DEPTH - 1

    int_lvls = list(range(DEPTH - 2, SMALL_MAX_LVL, -1))
    int_off = {}
    off = 0
    for l in int_lvls:
        int_off[l] = off
        off += _R(l)
    xint_d = nc.dram_tensor("xint", [128, off], BF16, kind="ExternalInput")
    n_units = _R(leaf) // (2 * T_TILE)
    xleaf_d = nc.dram_tensor("xleaf", [128, n_units * T_TILE], BF16,
                             kind="ExternalInput")
    small_cols = sum(_R(l) for l in range(SMALL_MAX_LVL + 1))
    xsmall_d = nc.dram_tensor("xsmall", [128, small_cols], BF16,
                              kind="ExternalInput")
    wcat_d = nc.dram_tensor("wcat", [128, 5 * HID], BF16, kind="ExternalInput")
    w_out_d = nc.dram_tensor("w_out", [HID, 2 * OUT_DIM], BF16,
                             kind="ExternalInput")
    out_d = nc.dram_tensor("out", [HID, B_LOCAL], F32, kind="ExternalOutput")
    if with_bias:
        bias_d = nc.dram_tensor("biases", [HID, 6], F32, kind="ExternalInput")
    if with_mask:
        total_z = sum(_R(l) for l in range(DEPTH))
        mask_d = nc.dram_tensor("mask_bc", [HID, total_z], BF16,
                                kind="ExternalInput")
        mask_off = {}
        moff = 0
        for l in range(DEPTH):
            mask_off[l] = moff
            moff += _R(l)

    from contextlib import ExitStack
    with tile.TileContext(nc) as tc, ExitStack() as stack:
        consts = stack.enter_context(tc.tile_pool(name="consts", bufs=1))
        hpool = stack.enter_context(tc.tile_pool(name="hpool", bufs=1))
        xpool = stack.enter_context(tc.tile_pool(name="xpool", bufs=6))
        apool = stack.enter_context(tc.tile_pool(name="apool", bufs=5))
        tpool = stack.enter_context(tc.tile_pool(name="tpool", bufs=4))
        pspool = stack.enter_context(tc.tile_pool(name="pspool", bufs=1,
                                                  space="PSUM"))
        opool = stack.enter_context(tc.tile_pool(name="opool", bufs=1))

        # scratch for PE warm-up / keep-alive dummy matmuls (HAM stays 8/8)
        scratch = consts.tile([128, 512], BF16, name="scratch", tag="scratch")
        nc.gpsimd.memset(scratch, 0)

        wcat_sb = consts.tile([128, 5 * HID], BF16, name="wcat_sb",
                              tag="wcat_sb")
        nc.sync.dma_start(out=wcat_sb, in_=wcat_d.ap())
        w_hr = wcat_sb[:, 0 * HID:1 * HID]
        w_hz = wcat_sb[:, 1 * HID:2 * HID]   # negated on host
        w_hn = wcat_sb[:, 2 * HID:3 * HID]
        wx = wcat_sb[:, 3 * HID:4 * HID]   # [w_ir; w_ir; -w_iz; w_in]
        wl = wcat_sb[:, 4 * HID:5 * HID]   # [-w_iz; -w_iz; 2*w_in; 2*w_in]
        w_out = consts.tile([HID, 2 * OUT_DIM], BF16, name="w_out_sb",
                            tag="w_out_sb")
        xsmall = consts.tile([128, small_cols], BF16, name="xsmall",
                             tag="xsmall")
        small_off = {}
        soff = 0
        for l in range(SMALL_MAX_LVL, -1, -1):
            small_off[l] = soff
            soff += _R(l)
        if with_bias:
            bias_sb = consts.tile([HID, 6], F32, name="bias_sb", tag="bias_sb")
            nc.sync.dma_start(out=bias_sb, in_=bias_d.ap())
            b_r = bias_sb[:, 0:1]      # b_ir + b_hr
            b_zneg = bias_sb[:, 1:2]   # -(b_iz + b_hz)
            b_n = bias_sb[:, 2:3]      # b_in + b_hn
            b_out = bias_sb[:, 3:4]
            b_lz = bias_sb[:, 4:5]     # -b_iz       (leaf z)
            b_ln = bias_sb[:, 5:6]     # 2*b_in      (leaf n, sigmoid form)

        h_lvl = [hpool.tile([HID, _R(l)], BF16, name=f"h_{l}", tag=f"h_{l}")
                 for l in range(DEPTH)]

        def mask_mul(view, lvl, col0, width):
            m_sb = tpool.tile([HID, width], BF16, name="m_sb", tag="m_sb")
            nc.sync.dma_start(
                out=m_sb,
                in_=mask_d.ap()[:, mask_off[lvl] + col0:
                                mask_off[lvl] + col0 + width])
            nc.vector.tensor_mul(view, view, m_sb)

        def kw_b(b):
            return dict(bias=b) if with_bias else {}

        def warm(n=128):
            """Keep-alive dummy matmul into psr[:, 0:n] (contents are dead
            between the r-ACT read and the next unit's overwrite). Full
            K=128 so the HAM activity monitor counts it."""
            ps = pspool.tile([HID, 2048], F32, name="ps_warm", tag="psr")
            nc.tensor.matmul(ps[:, 0:n], scratch[:, 0:HID],
                             scratch[:, 0:n], start=True, stop=True)

        # ---------------- leaf units ----------------
        # unit k covers tiles A = cols [kT,(k+1)T) and B = half + same.
        # All leaf gates are sigmoids (n via doubled weights): ONE [2048]
        # sigmoid per unit, alternating between the psr and psZN tiles.
        def leaf_unit(k):
            T = T_TILE
            xp = xpool.tile([128, T], BF16, name="xp_leaf", tag="xp")
            nc.sync.dma_start(out=xp, in_=xleaf_d.ap()[:, k * T:(k + 1) * T])
            zn = apool.tile([HID, 4 * T], BF16, name="zn_leaf", tag="act")
            if k % 2 == 0:
                ps = pspool.tile([HID, 2048], F32, name="psL", tag="psr")
            else:
                ps = pspool.tile([HID, 2048], F32, name="psLZ", tag="psZN")
            # strips of xp are [xA; xB; xA; xB] -> [zA | zB | nA | nB]
            for s in range(4):
                nc.tensor.matmul(ps[:, s * T:(s + 1) * T],
                                 wl[32 * s:32 * (s + 1)],
                                 xp[32 * s:32 * (s + 1)],
                                 start=True, stop=True,
                                 tile_position=(32 * s, 0))
            if not with_bias:
                nc.scalar.activation(zn, ps, SIGM)
            else:
                nc.scalar.activation(zn[:, 0:2 * T], ps[:, 0:2 * T], SIGM,
                                     bias=b_lz)
                nc.scalar.activation(zn[:, 2 * T:4 * T], ps[:, 2 * T:4 * T],
                                     SIGM, bias=b_ln)
            # h = zz * (2*sig_n - 1)
            u0 = tpool.tile([HID, 2 * T], BF16, name="u0_leaf", tag="u")
            nc.vector.tensor_scalar(u0, zn[:, 2 * T:4 * T], 2.0, -1.0,
                                    MULT, ADD)
            hv = h_lvl[leaf]
            half = _R(leaf) // 2
            hv2 = hv.rearrange("p (g f) -> p g f", g=2)[:, :, k * T:(k + 1) * T]
            zzv = zn[:, 0:2 * T].rearrange("p (g f) -> p g f", g=2)
            u0v = u0.rearrange("p (g f) -> p g f", g=2)
            nc.vector.tensor_mul(hv2, zzv, u0v)
            if with_mask:
                mask_mul(hv[:, k * T:(k + 1) * T], leaf, k * T, T)
                mask_mul(hv[:, half + k * T:half + (k + 1) * T], leaf,
                         half + k * T, T)

        # ------- interior pair (2 tiles of T parents), skewed emission ----
        # PSUM regions are bank-aligned at [512*s : 512*s + T], T <= 512.
        def pair_front(l, j, T):
            t0 = 2 * j * T
            ch = h_lvl[l + 1]
            R = _R(l)
            if l > SMALL_MAX_LVL:
                xpA = xpool.tile([128, T], BF16, name="xp_intA", tag="xp")
                nc.sync.dma_start(
                    out=xpA, in_=xint_d.ap()[:, int_off[l] + t0:
                                             int_off[l] + t0 + T])
                xpB = xpool.tile([128, T], BF16, name="xp_intB", tag="xp")
                nc.sync.dma_start(
                    out=xpB, in_=xint_d.ap()[:, int_off[l] + t0 + T:
                                             int_off[l] + t0 + 2 * T])
            else:
                xpA = xsmall[:, small_off[l] + t0:small_off[l] + t0 + T]
                xpB = xsmall[:, small_off[l] + t0 + T:small_off[l] + t0 + 2 * T]
            cs = tpool.tile([HID, 2 * T], BF16, name="cs", tag="cs",
                            padded_shape=[HID, 2 * T_TILE])
            nc.vector.tensor_add(cs, ch[:, t0:t0 + 2 * T],
                                 ch[:, R + t0:R + t0 + 2 * T])
            # psr regions: [rlA | rrA | rlB | rrB] at banks 0..3
            psr = pspool.tile([HID, 2048], F32, name="psr", tag="psr")
            for i, xp in enumerate((xpA, xpB)):
                for s in range(2):
                    o = (2 * i + s) * 512
                    nc.tensor.matmul(psr[:, o:o + T],
                                     wx[32 * s:32 * (s + 1)],
                                     xp[32 * s:32 * (s + 1)],
                                     start=True, stop=False,
                                     tile_position=(32 * s, 0))
                nc.tensor.matmul(psr[:, 2 * i * 512:2 * i * 512 + T], w_hr,
                                 ch[:, t0 + i * T:t0 + (i + 1) * T],
                                 start=False, stop=True)
                nc.tensor.matmul(psr[:, (2 * i + 1) * 512:
                                      (2 * i + 1) * 512 + T], w_hr,
                                 ch[:, R + t0 + i * T:R + t0 + (i + 1) * T],
                                 start=False, stop=True)
            r_sb = apool.tile([HID, 4 * T], BF16, name="r_sb", tag="act")
            psr_v = psr.rearrange("p (g f) -> p g f", g=4)[:, :, 0:T]
            r_v = r_sb.rearrange("p (g f) -> p g f", g=4)
            nc.scalar.activation(r_v, psr_v, SIGM,
                                 **kw_b(b_r if with_bias else None))
            # t2 = r * h_child (two-region child APs, one TT per tile)
            t2 = tpool.tile([HID, 4 * T], BF16, name="t2", tag="t2",
                            padded_shape=[HID, 4 * T_TILE])
            ch2 = ch.rearrange("p (g f) -> p g f", g=2)
            for i in range(2):
                sl = slice(2 * i * T, (2 * i + 2) * T)
                nc.vector.tensor_mul(
                    t2[:, sl].rearrange("p (g f) -> p g f", g=2),
                    r_sb[:, sl].rearrange("p (g f) -> p g f", g=2),
                    ch2[:, :, t0 + i * T:t0 + (i + 1) * T])
            return dict(l=l, t0=t0, T=T, cs=cs, t2=t2, xpA=xpA, xpB=xpB)

        def pair_back(st):
            l, t0, T, cs, t2 = st["l"], st["t0"], st["T"], st["cs"], st["t2"]
            # psZN regions: [zA | zB | nA | nB] at banks 0..3
            ps = pspool.tile([HID, 2048], F32, name="psZN", tag="psZN")
            for i, xp in enumerate((st["xpA"], st["xpB"])):
                nc.tensor.matmul(ps[:, i * 512:i * 512 + T],
                                 wx[64:96], xp[64:96],
                                 start=True, stop=False,
                                 tile_position=(64, 0))
                nc.tensor.matmul(ps[:, (2 + i) * 512:(2 + i) * 512 + T],
                                 wx[96:128], xp[96:128],
                                 start=True, stop=False,
                                 tile_position=(96, 0))
            for i in range(2):
                nc.tensor.matmul(ps[:, i * 512:i * 512 + T], w_hz,
                                 cs[:, i * T:(i + 1) * T],
                                 start=False, stop=True)
                nc.tensor.matmul(ps[:, (2 + i) * 512:(2 + i) * 512 + T], w_hn,
                                 t2[:, 2 * i * T:(2 * i + 1) * T],
                                 start=False, stop=False)
                nc.tensor.matmul(ps[:, (2 + i) * 512:(2 + i) * 512 + T], w_hn,
                                 t2[:, (2 * i + 1) * T:(2 * i + 2) * T],
                                 start=False, stop=True)
            zn = apool.tile([HID, 4 * T], BF16, name="zn_int", tag="act")
            ps_v = ps.rearrange("p (g f) -> p g f", g=4)
            nc.scalar.activation(
                zn[:, 0:2 * T].rearrange("p (g f) -> p g f", g=2),
                ps_v[:, 0:2, 0:T], SIGM,
                **kw_b(b_zneg if with_bias else None))
            nc.scalar.activation(
                zn[:, 2 * T:4 * T].rearrange("p (g f) -> p g f", g=2),
                ps_v[:, 2:4, 0:T], TANH,
                **kw_b(b_n if with_bias else None))
            # h = cs + zz*(n - cs)
            u = tpool.tile([HID, 2 * T], BF16, name="u_sb", tag="u",
                           padded_shape=[HID, 2 * T_TILE])
            nc.vector.tensor_sub(u, zn[:, 2 * T:4 * T], cs)
            v = tpool.tile([HID, 2 * T], BF16, name="v_sb", tag="v",
                           padded_shape=[HID, 2 * T_TILE])
            nc.vector.tensor_mul(v, zn[:, 0:2 * T], u)
            nc.vector.tensor_add(h_lvl[l][:, t0:t0 + 2 * T], v, cs)
            if with_mask:
                mask_mul(h_lvl[l][:, t0:t0 + 2 * T], l, t0, 2 * T)

        # ---------------- tail solo unit (one tile of R parents) --------
        def make_cs(l):
            """cs for level l, fused into the producer's DVE stream."""
            Rp = _R(l)
            ch = h_lvl[l + 1]
            cs = tpool.tile([HID, Rp], BF16, name="cs_t", tag="cs")
            nc.vector.tensor_add(cs, ch[:, 0:Rp], ch[:, Rp:2 * Rp])
            return cs

        def solo_unit(l, cs, xp_pre=None):
            Rp = _R(l)
            ch = h_lvl[l + 1]
            if xp_pre is not None:
                xp = xp_pre
            elif l > SMALL_MAX_LVL:
                xp = xpool.tile([128, Rp], BF16, name="xp_tail", tag="xp")
                nc.sync.dma_start(
                    out=xp, in_=xint_d.ap()[:, int_off[l]:int_off[l] + Rp])
            else:
                xp = xsmall[:, small_off[l]:small_off[l] + Rp]
            psr = pspool.tile([HID, 2048], F32, name="psr_t", tag="psr")
            # z into psr bank 2 (off the critical chain, only needs cs)
            def warm_t():
                nc.tensor.matmul(psr[:, 1536:1792], scratch[:, 0:HID],
                                 scratch[:, 0:256], start=True, stop=True)

            nc.tensor.matmul(psr[:, 1024:1024 + Rp], wx[64:96], xp[64:96],
                             start=True, stop=False, tile_position=(64, 0))
            nc.tensor.matmul(psr[:, 1024:1024 + Rp], w_hz, cs,
                             start=False, stop=True)
            warm_t()
            # r into banks 0 / 1
            for s, o in ((0, 0), (1, 512)):
                nc.tensor.matmul(psr[:, o:o + Rp],
                                 wx[32 * s:32 * (s + 1)],
                                 xp[32 * s:32 * (s + 1)],
                                 start=True, stop=False,
                                 tile_position=(32 * s, 0))
            nc.tensor.matmul(psr[:, 0:Rp], w_hr, ch[:, 0:Rp],
                             start=False, stop=True)
            nc.tensor.matmul(psr[:, 512:512 + Rp], w_hr, ch[:, Rp:2 * Rp],
                             start=False, stop=True)
            rz = apool.tile([HID, 3 * Rp], BF16, name="rz_tail", tag="act")
            psr_v = psr.rearrange("p (g f) -> p g f", g=4)[:, 0:3, 0:Rp]
            rz_v = rz.rearrange("p (g f) -> p g f", g=3)
            if with_bias:
                # biases differ between r and z: separate ACTs
                nc.scalar.activation(
                    rz[:, 0:2 * Rp].rearrange("p (g f) -> p g f", g=2),
                    psr.rearrange("p (g f) -> p g f", g=4)[:, 0:2, 0:Rp],
                    SIGM, bias=b_r)
                nc.scalar.activation(rz[:, 2 * Rp:3 * Rp],
                                     psr[:, 1024:1024 + Rp], SIGM,
                                     bias=b_zneg)
            else:
                nc.scalar.activation(rz_v, psr_v, SIGM)
            warm_t()
            t2 = tpool.tile([HID, 2 * Rp], BF16, name="t2_t", tag="t2")
            ch2 = ch.rearrange("p (g f) -> p g f", g=2)[:, :, 0:Rp]
            nc.vector.tensor_mul(t2.rearrange("p (g f) -> p g f", g=2),
                                 rz[:, 0:2 * Rp].rearrange(
                                     "p (g f) -> p g f", g=2), ch2)
            psN = pspool.tile([HID, 2048], F32, name="psN_t", tag="psZN")
            nc.tensor.matmul(psN[:, 0:Rp], wx[96:128], xp[96:128],
                             start=True, stop=False, tile_position=(96, 0))
            nc.tensor.matmul(psN[:, 0:Rp], w_hn, t2[:, 0:Rp],
                             start=False, stop=False)
            nc.tensor.matmul(psN[:, 0:Rp], w_hn, t2[:, Rp:2 * Rp],
                             start=False, stop=True)
            zn = apool.tile([HID, Rp], BF16, name="zn_tail", tag="act")
            nc.scalar.activation(zn, psN[:, 0:Rp], TANH,
                                 **kw_b(b_n if with_bias else None))
            u = tpool.tile([HID, Rp], BF16, name="u_t", tag="u")
            v = tpool.tile([HID, Rp], BF16, name="v_t", tag="v")
            nc.vector.tensor_sub(u, zn, cs)
            nc.vector.tensor_mul(v, rz[:, 2 * Rp:3 * Rp], u)
            nc.vector.tensor_add(h_lvl[l][:, 0:Rp], v, cs)
            if with_mask:
                mask_mul(h_lvl[l][:, 0:Rp], l, 0, Rp)
            return make_cs(l - 1) if l > 0 else None

        # ---------------- emission ----------------
        for k in range(n_units):
            if k == 2:
                nc.sync.dma_start(out=xsmall, in_=xsmall_d.ap())
                nc.sync.dma_start(out=w_out, in_=w_out_d.ap())
            leaf_unit(k)

        # pair levels 13..9; levels 10/9 use smaller tiles (more units) to
        # keep the pipeline deep; [0,H,1,H+1] order per level so fronts
        # unblock while the level above is still draining.
        T_OF = {13: 512, 12: 512, 11: 512, 10: 256, 9: 128}

        def sched_for(l):
            n = _R(l) // T_OF[l] // 2
            h = max(n // 2, 1)
            order = []
            for a in range(h):
                order.append(a)
                if a + h < n:
                    order.append(a + h)
            return [(l, j) for j in order]

        schedule = []
        for l in range(13, 8, -1):
            schedule += sched_for(l)

        def blocks(front, pending):
            # does `pending`'s h-write intersect `front`'s child reads?
            if pending is None:
                return False
            lf, jf = front
            if pending["l"] != lf + 1:
                return False
            Tf = T_OF[lf]
            t0f = 2 * jf * Tf
            R = _R(lf)
            w0, w1 = pending["t0"], pending["t0"] + 2 * pending["T"]
            for a, b in ((t0f, t0f + 2 * Tf), (R + t0f, R + t0f + 2 * Tf)):
                if a < w1 and w0 < b:
                    return True
            return False

        prev = None
        for (l, j) in schedule:
            if blocks((l, j), prev):
                pair_back(prev)
                prev = None
            st = pair_front(l, j, T_OF[l])
            if prev is not None:
                pair_back(prev)
            prev = st
        pair_back(prev)

        # tail levels 8..0 (cs fused into the producing unit's DVE stream)
        xp8 = xpool.tile([128, _R(8)], BF16, name="xp8", tag="xp")
        nc.sync.dma_start(out=xp8, in_=xint_d.ap()[:, int_off[8]:
                                                   int_off[8] + _R(8)])
        cs = make_cs(8)
        for l in range(8, -1, -1):
            cs = solo_unit(l, cs, xp8 if l == 8 else None)

        # ---------------- output head ----------------
        ps_out = pspool.tile([HID, B_LOCAL], F32, name="ps_out", tag="psZN")
        nc.tensor.matmul(ps_out, w_out, h_lvl[0], start=True, stop=True)
        out_sb = opool.tile([HID, B_LOCAL], F32, name="out_sb", tag="out_sb")
        if with_bias:
            nc.scalar.activation(out_sb, ps_out,
                                 mybir.ActivationFunctionType.Identity,
                                 bias=b_out)
        else:
            nc.scalar.copy(out_sb, ps_out)
        nc.sync.dma_start(out=out_d.ap(), in_=out_sb)

    nc.compile()
    return nc


def host_prep(inputs, with_mask=False, with_bias=False):
    t = np.ascontiguousarray(np.asarray(inputs["targets"], np.float32))
    N = t.shape[0]
    assert N == 2**DEPTH - 1 and t.shape[2] == IN_DIM
    leaf = DEPTH - 1

    xt = np.ascontiguousarray(t.transpose(2, 0, 1)).astype(BF16NP)
    revs = {l: _bitrev(l) for l in range(DEPTH)}

    def plain_t(w):
        return np.ascontiguousarray(np.asarray(w, np.float32).T).astype(BF16NP)

    w_ir = plain_t(inputs["W_ir"])
    w_izn = np.ascontiguousarray(
        -np.asarray(inputs["W_iz"], np.float32).T).astype(BF16NP)
    w_in = plain_t(inputs["W_in"])
    w_in2 = np.ascontiguousarray(
        2.0 * np.asarray(inputs["W_in"], np.float32).T).astype(BF16NP)
    w_out = np.ascontiguousarray(
        np.concatenate([np.asarray(inputs["W_mu"], np.float32),
                        np.asarray(inputs["W_lv"], np.float32)],
                       axis=0).T).astype(BF16NP)

    wcat = np.zeros((128, 5 * HID), BF16NP)
    wcat[:, 0 * HID:1 * HID] = plain_t(inputs["W_hr"])
    wcat[:, 1 * HID:2 * HID] = np.ascontiguousarray(
        -np.asarray(inputs["W_hz"], np.float32).T).astype(BF16NP)
    wcat[:, 2 * HID:3 * HID] = plain_t(inputs["W_hn"])
    for i, wsrc in enumerate((w_ir, w_ir, w_izn, w_in)):
        wcat[32 * i:32 * (i + 1), 3 * HID:4 * HID] = wsrc
    for i, wsrc in enumerate((w_izn, w_izn, w_in2, w_in2)):
        wcat[32 * i:32 * (i + 1), 4 * HID:5 * HID] = wsrc

    shared = dict(wcat=wcat, w_out=w_out)
    if with_bias:
        b = {k: np.asarray(inputs[k], np.float32) for k in
             ("b_ir", "b_hr", "b_iz", "b_hz", "b_in", "b_hn", "b_mu", "b_lv")}
        bias = np.zeros((HID, 6), np.float32)
        bias[:, 0] = b["b_ir"] + b["b_hr"]
        bias[:, 1] = -(b["b_iz"] + b["b_hz"])
        bias[:, 2] = b["b_in"] + b["b_hn"]
        bias[:128, 3] = np.concatenate([b["b_mu"], b["b_lv"]])
        bias[:, 4] = -b["b_iz"]
        bias[:, 5] = 2.0 * b["b_in"]
        shared["biases"] = bias

    int_lvls = list(range(DEPTH - 2, SMALL_MAX_LVL, -1))
    n_units = _R(leaf) // (2 * T_TILE)

    in_maps = []
    for c in range(N_CORES):
        b0 = c * B_LOCAL
        xc = xt[:, :, b0:b0 + B_LOCAL]
        xl = {}
        for l in range(DEPTH):
            start = 2**l - 1
            blk = xc[:, start + revs[l], :]
            xl[l] = np.ascontiguousarray(blk.reshape(IN_DIM, _R(l)))

        xint = np.concatenate([np.tile(xl[l], (4, 1)) for l in int_lvls],
                              axis=1)
        half = _R(leaf) // 2
        lblocks = []
        for k in range(n_units):
            xA = xl[leaf][:, k * T_TILE:(k + 1) * T_TILE]
            xB = xl[leaf][:, half + k * T_TILE:half + (k + 1) * T_TILE]
            lblocks.append(np.concatenate([xA, xB, xA, xB], axis=0))
        xleaf = np.concatenate(lblocks, axis=1)
        xsmall = np.concatenate([np.tile(xl[l], (4, 1))
                                 for l in range(SMALL_MAX_LVL, -1, -1)],
                                axis=1)
        m = dict(shared)
        m["xint"] = np.ascontiguousarray(xint)
        m["xleaf"] = np.ascontiguousarray(xleaf)
        m["xsmall"] = np.ascontiguousarray(xsmall)
        if with_mask:
            mk = np.asarray(inputs["mask"], np.float32)[:, b0:b0 + B_LOCAL]
            mblocks = []
            for l in range(DEPTH):
                start = 2**l - 1
                mblocks.append(mk[start + revs[l], :].reshape(1, _R(l)))
            mcat = np.concatenate(mblocks, axis=1)
            m["mask_bc"] = np.ascontiguousarray(
                np.broadcast_to(mcat, (HID, mcat.shape[1]))).astype(BF16NP)
        in_maps.append(m)
    return in_maps


_PROGRAM_CACHE = {}


def _get_program(with_mask, with_bias):
    key = (with_mask, with_bias)
    if key not in _PROGRAM_CACHE:
        _PROGRAM_CACHE[key] = build_program(with_mask=with_mask,
                                            with_bias=with_bias)
    return _PROGRAM_CACHE[key]


def run_on_device(inputs, trace=False, **trace_kw):
    with_mask = not np.all(np.asarray(inputs["mask"]) == 1.0)
    with_bias = any(
        np.any(np.asarray(inputs[k]) != 0.0)
        for k in ("b_ir", "b_hr", "b_iz", "b_hz", "b_in", "b_hn",
                  "b_mu", "b_lv"))
    nc = _get_program(with_mask, with_bias)
    in_maps = host_prep(inputs, with_mask=with_mask, with_bias=with_bias)
    res = bass_utils.run_bass_kernel_spmd(
        nc, in_maps, core_ids=list(range(N_CORES)), trace=trace, **trace_kw)
    mu = np.zeros((BATCH, OUT_DIM), np.float32)
    lv = np.zeros((BATCH, OUT_DIM), np.float32)
    for c in range(N_CORES):
        o = res.results[c]["out"]
        mu[c * B_LOCAL:(c + 1) * B_LOCAL] = o[:OUT_DIM].T
        lv[c * B_LOCAL:(c + 1) * B_LOCAL] = o[OUT_DIM:].T
    return (mu, lv), res


def kernel(**inputs):
    (mu, lv), _ = run_on_device(inputs)
    return mu, lv
